# revision 46
# baseline (speedup 1.0000x reference)
"""Trainium2 Bass kernel for nn_ExpandedResolventFMNet.

Mathematical reformulation (validated in fp64 against the jax reference):

The reference builds kron(A.T, My) [8192x4096], its Gram [4096^2], resolvent
kron masks, and solves a dense 4096x4096 system.  All of that collapses:

  first        = kron(A A^T, G),              G = My^T My
  second       = kron-sum of 64x64 factors; the device runs the transposed
                 system in Y = W^T:
  M'(Y)        = G Y S~ + sum_d DdT * (G (DdT * Y)),   C = Y Mx^T
  S~           = Mx^T (A A^T) Mx
  rhs R'       = G By A^T Mx      (uses My^T My By = G By)
  DdT          = resolvent-mask difference matrices (64x64), * = Hadamard

solved by pipelined PCG with the exact-kron preconditioner P^-1 x = Gi x Si
(Gi ~= G^-1, Si ~= S~^-1 via on-device bf16 Newton-Schulz).

Implementation decisions (each backed by a trace or an offline numerics
study; baseline 163.5us -> 83.6us at rel_err 7.4e-3 vs the 2e-2 gate):
 - No collectives: each of the 8 cores runs the full problem redundantly
   and core 0 is read back.  The sharded baseline lost 59us to a 37us
   first-collective barrier + 2x11us AllReduce.
 - Projections in bf16 (input rounding amplifies only ~1.3x through the
   solve; fp32 matmuls are 4 cycles/row vs 1 for bf16).
 - Gram chain and rhs in float32r (single-pass, ~12-bit mantissa measured
   1.25e-4 median rounding on HW); PCG matvec and preconditioner
   applications also f32r: bf16 applications inside the pipelined
   s-recurrence drift to 1.4e-2 (nonlinear rounding compounds in the
   recurrence), f32r holds ~6e-3.
 - gpsimd runs ONLY partition_all_reduce: mixing op families on the Q7
   (elementwise / memset / SWDGE DMA) forces ~7us library reloads.
 - Big inputs are host-pre-arranged to [125, 40*dim] so partition p owns
   vertex rows 40p..40p+39 as one contiguous line; the DMA then coalesces
   to full-line descriptors (a strided rearrange view ran at ~60GB/s).
   6 DMAs total, x-side first on both HWDGE rings (>8 in-flight DMAs
   alias the 8 completion lanes and chain unrelated waits).
 - The y accumulation carries a negated sign (alpha reuses the negated
   <p,q> dot, saving vector ops); the sign is folded into a host-negated
   Mx^T at the output matmul.
 - A warm-up matmul burst during the DMA window ramps the PE p-state.
"""

import numpy as np
import ml_dtypes

import concourse.bacc as bacc
import concourse.mybir as mybir
from concourse.bass_isa import ReduceOp
from concourse.bass_utils import run_bass_kernel_spmd
from concourse.tile import TileContext

F32 = mybir.dt.float32
F32R = mybir.dt.float32r
BF16 = mybir.dt.bfloat16
K = 64          # spectral basis size
C = 128         # feature channels
V = 5000        # vertices
CHUNK = 125     # v-contraction tile (partition dim)
NCH = V // CHUNK                 # 40 chunks
# Single core: the solve is one serial latency-bound chain, and running it
# replicated on all 8 cores makes each core's 3.84MB input pull contend for
# chip HBM (measured ~60GB/s/core vs 358 alone) with zero benefit — only
# core 0's output is read.  One core owns the full ~358GB/s.
N_CORES = 1
N_ITERS = 6
NEWTON_STEPS_S = 6
NEWTON_STEPS_G = 3
SQRT_LMBDA = 10.0

SHARD = False   # kept for test.py compat; ignored (always replicated)

_PROGRAM_CACHE = {}


def build_program(shard: bool = False, debug: bool = False):
    nc = bacc.Bacc("TRN2", num_devices=N_CORES)
    dbg = {}
    if debug:
        for nm, shp in (("d_at", [C, K]), ("d_byt", [C, K]), ("d_st", [K, K]),
                        ("d_gi", [K, K]), ("d_si", [K, K]), ("d_r", [K, K]),
                        ("d_g", [K, K]), ("d_z0", [K, K])):
            dbg[nm] = nc.dram_tensor(nm, shp, F32, kind="ExternalOutput")

    # big inputs are host-pre-arranged to [CHUNK, NCH*dim]: partition p owns
    # vertex rows 40p..40p+39 as one contiguous 10KB/5KB line, so each DMA
    # coalesces into full-line descriptors (the (p n) c rearrange view kept
    # 256B descriptors and ran at ~60GB/s).
    fx_d = nc.dram_tensor("fx", [CHUNK, NCH * C], BF16, kind="ExternalInput")
    fy_d = nc.dram_tensor("fy", [CHUNK, NCH * C], BF16, kind="ExternalInput")
    pxT_d = nc.dram_tensor("pxT", [CHUNK, NCH * K], BF16, kind="ExternalInput")
    pyT_d = nc.dram_tensor("pyT", [CHUNK, NCH * K], BF16, kind="ExternalInput")
    small_d = nc.dram_tensor("small", [K, 4 * K], F32, kind="ExternalInput")
    ev_d = nc.dram_tensor("ev", [1, 2 * K], F32, kind="ExternalInput")
    out_d = nc.dram_tensor("out", [K, K], F32, kind="ExternalOutput")

    with TileContext(nc) as tc:
        with (
            tc.tile_pool(name="big", bufs=1) as bp,
            tc.tile_pool(name="persist", bufs=1) as sp,
            tc.tile_pool(name="work", bufs=2) as wp,
            tc.tile_pool(name="psum", bufs=2, space="PSUM") as pp,
        ):
            # rotating psum tags: 3 tags x bufs=2 -> 6 banks (+2 proj acc = 8)
            _ps_state = {"i": 0}

            def ps_tile(shape):
                i = _ps_state["i"]
                _ps_state["i"] += 1
                return pp.tile(shape, F32, tag=f"ps{i % 3}", name=f"pst{i}")

            # ---------------- input DMA ------------------------------------
            # gpsimd runs ONLY partition_all_reduce in this kernel: every
            # other op family (elementwise, memset, affine_select, SWDGE DMA)
            # forces a Q7 library reload costing ~7us.  Identity comes from
            # the host; y-side DMAs ride the scalar-engine HWDGE ring so x
            # and y stream in parallel.
            # all four [K,K] small tensors ride in ONE packed DMA: every
            # extra DMA costs a semaphore lane and queue slot, and lane
            # aliasing was observed to chain projection matmuls behind
            # unrelated late small DMAs.
            small_s = sp.tile([K, 4 * K], F32)
            ev_t = sp.tile([1, 2 * K], F32)
            nc.sync.dma_start(ev_t, ev_d[:, :])
            nc.sync.dma_start(small_s, small_d[:, :])
            my_s = small_s[:, 0:K]
            mx_s = small_s[:, K:2 * K]
            id64 = small_s[:, 2 * K:3 * K]
            mxTn_s = small_s[:, 3 * K:4 * K]

            # partition p holds vertex rows [p*NCH, (p+1)*NCH) — contiguous
            # 10KB/5KB per partition line -> near-line-rate DMA.  Chunk n of
            # the V-contraction is rows {p*NCH + n}; any partition of V into
            # chunks is valid for the sum.
            # 6 DMAs total: more than 8 in flight aliases the shared HWDGE
            # completion lanes and chains projections behind unrelated DMAs.
            # x-side tensors lead on BOTH rings (the rings share one ~358GB/s
            # HBM pipe, so queue order sets completion order).
            fx_t = bp.tile([CHUNK, NCH * C], BF16)
            fy_t = bp.tile([CHUNK, NCH * C], BF16)
            pxT_t = bp.tile([CHUNK, NCH * K], BF16)
            pyT_t = bp.tile([CHUNK, NCH * K], BF16)
            nc.sync.dma_start(fx_t, fx_d[:, :])
            nc.scalar.dma_start(pxT_t, pxT_d[:, :])
            nc.scalar.dma_start(pyT_t, pyT_d[:, :])
            nc.sync.dma_start(fy_t, fy_d[:, :])

            ones_row = sp.tile([1, K], F32)
            nc.vector.memset(ones_row, 1.0)

            # PE warm-up: sustained matmuls ramp the PE clock out of its low
            # p-state (cold bf16 matmuls run ~2x slow); runs during the DMA
            # window on garbage data.
            warm = sp.tile([CHUNK, 512], BF16)
            nc.vector.memset(warm, 1.0)
            for wi in range(8):
                wp_p = pp.tile([128, 512], F32, tag=f"ps{wi % 3}",
                               name=f"warm{wi}")
                nc.tensor.matmul(wp_p, warm[:, 0:128], warm)

            # ------- pre-work that only needs the small tensors -------------
            evmax = sp.tile([1, 1], F32)
            nc.vector.tensor_reduce(evmax, ev_t, mybir.AxisListType.X,
                                    mybir.AluOpType.max)
            evrec = sp.tile([1, 1], F32)
            nc.vector.reciprocal(evrec, evmax)
            t_t = sp.tile([1, 2 * K], F32)
            nc.vector.tensor_scalar_mul(t_t, ev_t, evrec)
            tp1 = sp.tile([1, 2 * K], F32)
            nc.vector.tensor_scalar_add(tp1, t_t, 1.0)
            im_t = sp.tile([1, 2 * K], F32)
            nc.vector.reciprocal(im_t, tp1)
            sq_t = sp.tile([1, 2 * K], F32)
            nc.scalar.sqrt(sq_t, t_t)
            re_t = sp.tile([1, 2 * K], F32)
            nc.vector.tensor_mul(re_t, sq_t, im_t)
            nc.vector.tensor_scalar_mul(re_t, re_t, SQRT_LMBDA)
            nc.vector.tensor_scalar_mul(im_t, im_t, SQRT_LMBDA)

            # D-mask matmuls are emitted later (after S~) so they never sit
            # in front of the projections on the in-order tensor queue; the
            # builder closure runs once its emission point is reached.
            def emit_dmasks():
                d_s = []
                for idx, src in enumerate((re_t, im_t)):
                    pa = ps_tile([K, K])
                    nc.tensor.matmul(pa, src[0:1, K:2 * K], ones_row)
                    pb = ps_tile([K, K])
                    nc.tensor.matmul(pb, ones_row, src[0:1, 0:K])
                    ta = sp.tile([K, K], F32, tag=f"dta{idx}",
                                 name=f"dta{idx}")
                    nc.vector.tensor_copy(ta, pa)
                    dt = sp.tile([K, K], F32, tag=f"d{idx}t_s",
                                 name=f"d{idx}t_s")
                    nc.vector.tensor_sub(dt, ta, pb)
                    d_s.append(dt)
                d12 = sp.tile([K, 2 * K], F32)
                nc.vector.tensor_copy(d12[:, 0:K], d_s[0])
                nc.vector.tensor_copy(d12[:, K:2 * K], d_s[1])
                return d_s[0], d_s[1], d12

            # G = My^T My  (fp32; f32r copy for the solver)
            g_p = ps_tile([K, K])
            nc.tensor.matmul(g_p, my_s, my_s)
            g_s = sp.tile([K, K], F32)
            nc.vector.tensor_copy(g_s, g_p)
            g_bf = sp.tile([K, K], BF16)
            nc.scalar.copy(g_bf, g_p)
            g_r = sp.tile([K, K], F32R)
            nc.vector.tensor_copy(g_r, g_p)
            mx_r = sp.tile([K, K], F32R)
            nc.vector.tensor_copy(mx_r, mx_s)

            # Newton-Schulz inverse in bf16 (preconditioner values only need
            # ~1% accuracy).  X' = 2X - X S X, X0 = I / max_rowsum(|S|).
            _newton_out = {}

            def newton_inverse_bf(mat_bf, mat_f32, tag, steps):
                rs = sp.tile([K, 1], F32, tag=f"{tag}_rs", name=f"{tag}_rs")
                nc.vector.tensor_reduce(rs, mat_f32, mybir.AxisListType.X,
                                        mybir.AluOpType.add,
                                        apply_absolute_value=True)
                nc.gpsimd.partition_all_reduce(rs, rs, K, ReduceOp.max)
                al = sp.tile([K, 1], F32, tag=f"{tag}_al", name=f"{tag}_al")
                nc.vector.reciprocal(al, rs)
                x_bf = sp.tile([K, K], BF16, tag=f"{tag}_x0", name=f"{tag}_x0")
                nc.vector.tensor_scalar_mul(x_bf, id64, al)
                for it in range(steps):
                    t1 = ps_tile([K, K])
                    nc.tensor.matmul(t1, mat_bf, x_bf)        # S X (S sym)
                    t1s = wp.tile([K, K], BF16, tag=f"{tag}_t1s",
                                  name=f"{tag}_t1s")
                    nc.vector.tensor_copy(t1s, t1)
                    t2 = ps_tile([K, K])
                    nc.tensor.matmul(t2, x_bf, t1s)           # X (S X) (X sym)
                    xn = sp.tile([K, K], BF16, tag=f"{tag}_x{it + 1}",
                                 name=f"{tag}_x{it + 1}")
                    nc.vector.scalar_tensor_tensor(
                        xn, x_bf, 2.0, t2,
                        op0=mybir.AluOpType.mult,
                        op1=mybir.AluOpType.subtract)
                    x_bf = xn
                    yield
                # f32r copy for the solver-side applications
                x_r = sp.tile([K, K], F32R, tag=f"{tag}_xr", name=f"{tag}_xr")
                nc.vector.tensor_copy(x_r, x_bf)
                _newton_out[tag] = x_r

            # ---------------- x projection: A^T = fx^T pxT ------------------
            with tc.tile_pool(name="pacc", bufs=1, space="PSUM") as pacc:
                at_p = pacc.tile([C, K], F32)    # A^T  [C,K]
                byt_p = pacc.tile([C, K], F32)   # By^T [C,K]
                for n in range(NCH):
                    nc.tensor.matmul(at_p, fx_t[:, n * C:(n + 1) * C],
                                     pxT_t[:, n * K:(n + 1) * K],
                                     start=(n == 0), stop=(n == NCH - 1))
                at_r = sp.tile([C, K], F32R)
                nc.vector.tensor_copy(at_r, at_p)
                if debug:
                    nc.sync.dma_start(dbg["d_at"][:, :], at_r.bitcast(F32))

                # ---- S~ = Mx^T (A A^T) Mx (f32r, symmetric tricks) ----
                sa_p = ps_tile([K, K])
                nc.tensor.matmul(sa_p, at_r, at_r)          # A A^T
                sa_r = sp.tile([K, K], F32R)
                nc.scalar.copy(sa_r, sa_p)
                h1_p = ps_tile([K, K])
                nc.tensor.matmul(h1_p, sa_r, mx_r)          # S_A Mx (sym)
                h1_r = sp.tile([K, K], F32R)
                nc.scalar.copy(h1_r, h1_p)
                st_p = ps_tile([K, K])
                nc.tensor.matmul(st_p, mx_r, h1_r)          # Mx^T S_A Mx
                st_s = sp.tile([K, K], F32)
                nc.vector.tensor_copy(st_s, st_p)
                st_bf = sp.tile([K, K], BF16)
                nc.scalar.copy(st_bf, st_p)
                st_r = sp.tile([K, K], F32R)
                nc.vector.tensor_copy(st_r, st_p)
                if debug:
                    nc.sync.dma_start(dbg["d_st"][:, :], st_s)

                d1t_s, d2t_s, d12t_s = emit_dmasks()

                # ---- Newton-Si (bf16) + eager y projections + rhs chain ---
                ystate = {"n": 0}

                def emit_y(k):
                    for _ in range(k):
                        n = ystate["n"]
                        if n >= NCH:
                            return
                        ystate["n"] += 1
                        nc.tensor.matmul(byt_p, fy_t[:, n * C:(n + 1) * C],
                                         pyT_t[:, n * K:(n + 1) * K],
                                         start=(n == 0), stop=(n == NCH - 1))

                rhs_out = {}

                def emit_rhs():
                    """rhs R' = G By A^T Mx (f32r) — independent of Si."""
                    byt_r = sp.tile([C, K], F32R)
                    nc.vector.tensor_copy(byt_r, byt_p)
                    if debug:
                        nc.sync.dma_start(dbg["d_byt"][:, :],
                                          byt_r.bitcast(F32))
                    byat_p = ps_tile([K, K])
                    nc.tensor.matmul(byat_p, byt_r, at_r)   # By A^T
                    byat_r = sp.tile([K, K], F32R)
                    nc.scalar.copy(byat_r, byat_p)
                    w0_p = ps_tile([K, K])
                    nc.tensor.matmul(w0_p, g_r, byat_r)     # G By A^T (G sym)
                    w0_s = sp.tile([K, K], F32)
                    nc.vector.tensor_copy(w0_s, w0_p)
                    wt_p = ps_tile([K, K])
                    nc.tensor.transpose(wt_p, w0_s, id64)
                    wt_r = sp.tile([K, K], F32R)
                    nc.scalar.copy(wt_r, wt_p)
                    rp_p = ps_tile([K, K])
                    nc.tensor.matmul(rp_p, wt_r, mx_r)      # (G By A^T) Mx
                    r_t = sp.tile([K, K], F32)              # CG residual
                    nc.vector.tensor_copy(r_t, rp_p)
                    rhs_out["r_s"] = r_t

                # The two independent Newton chains (Gi, Si) are interleaved
                # step-by-step: Gi's serial latency hides inside Si's
                # cross-engine bubbles (both sit after the x-projections so
                # the gpsimd library load never stalls them).  The y
                # projections are emitted as one consecutive block afterwards
                # — back-to-back accumulation matmuls pipeline at ~64ns,
                # whereas interleaving Newton steps between them restarts the
                # PE pipeline at every switch — and the Newton chains' serial
                # tail hides the y-DMA arrival.
                gi_gen = newton_inverse_bf(g_bf, g_s, "gi", NEWTON_STEPS_G)
                si_gen = newton_inverse_bf(st_bf, st_s, "si", NEWTON_STEPS_S)
                alive = {gi_gen, si_gen}
                while alive:
                    for gen in (si_gen, gi_gen):
                        if gen in alive:
                            try:
                                next(gen)
                            except StopIteration:
                                alive.discard(gen)
                gi_r = _newton_out["gi"]
                si_r = _newton_out["si"]
                emit_y(NCH)
                emit_rhs()
                r_s = rhs_out["r_s"]

            if debug:
                nc.sync.dma_start(dbg["d_r"][:, :], r_s)
                gi_f = sp.tile([K, K], F32, tag="dbg_gi", name="dbg_gi")
                nc.vector.tensor_copy(gi_f, gi_r)
                nc.sync.dma_start(dbg["d_gi"][:, :], gi_f)
                si_f = sp.tile([K, K], F32, tag="dbg_si", name="dbg_si")
                nc.vector.tensor_copy(si_f, si_r)
                nc.sync.dma_start(dbg["d_si"][:, :], si_f)
                nc.sync.dma_start(dbg["d_g"][:, :], g_s)

            # ------- pipelined PCG (f32r matvec + precond applications) -----
            yn_s = sp.tile([K, K], F32)      # accumulates -y
            nc.vector.memset(yn_s, 0.0)
            p_s = sp.tile([K, K], F32)
            q_s = sp.tile([K, K], F32)
            s_s = sp.tile([K, K], F32)
            z_s = sp.tile([K, K], F32R)      # f32r so matvec mms read it
            u_s = sp.tile([K, 2 * K], F32R)  # stacked [D1T*z | D2T*z]

            def precond_psum(x_tile, tag):
                """P^-1 x in PSUM: (Gi x)^T = mm(lhsT=x_r, Gi); then * Si."""
                xb = wp.tile([K, K], F32R, tag=f"{tag}_xb", name=f"{tag}_xb")
                nc.scalar.copy(xb, x_tile)
                ut_p = ps_tile([K, K])
                nc.tensor.matmul(ut_p, xb, gi_r)
                ut_s = wp.tile([K, K], F32R, tag=f"{tag}_uts",
                               name=f"{tag}_uts")
                nc.scalar.copy(ut_s, ut_p)
                v_p = ps_tile([K, K])
                nc.tensor.matmul(v_p, ut_s, si_r)
                return v_p

            def matvec_z(tag):
                """w = M z into SBUF (reads z_s); f32r matmuls."""
                nc.vector.tensor_mul(u_s[:, 0:K], d1t_s, z_s)
                nc.vector.tensor_mul(u_s[:, K:2 * K], d2t_s, z_s)
                gzt_p = ps_tile([K, K])
                nc.tensor.matmul(gzt_p, z_s, g_r)         # (G z)^T
                gzt_s = wp.tile([K, K], F32R, tag="mv_gzts", name="mv_gzts")
                nc.vector.tensor_copy(gzt_s, gzt_p)
                t2_p = ps_tile([K, K])
                nc.tensor.matmul(t2_p, gzt_s, st_r)       # (G z) S~
                gu_p = ps_tile([K, 2 * K])
                nc.tensor.matmul(gu_p, g_r, u_s)          # G [u1|u2], one mm
                mm_s = wp.tile([K, 2 * K], F32, tag="mv_mm", name="mv_mm")
                nc.vector.tensor_mul(mm_s, d12t_s, gu_p)  # masked, both halves
                a1_s = wp.tile([K, K], F32, tag="mv_a1", name="mv_a1")
                nc.vector.tensor_add(a1_s, mm_s[:, 0:K], t2_p)
                w_s = wp.tile([K, K], F32, tag="mv_w", name="mv_w")
                nc.vector.tensor_add(w_s, a1_s, mm_s[:, K:2 * K])
                return w_s

            def dot_b(a_ap, b_ap, tag, scale=1.0):
                """<a,b>*scale broadcast to all partitions as [K,1] SBUF."""
                prod = wp.tile([K, K], F32, tag="dot_dm", name="dot_dm")
                acc = wp.tile([K, 1], F32, tag=f"{tag}_acc", name=f"{tag}_acc")
                nc.vector.scalar_tensor_tensor(
                    prod, a_ap, scale, b_ap,
                    op0=mybir.AluOpType.mult, op1=mybir.AluOpType.mult,
                    accum_out=acc)
                nc.gpsimd.partition_all_reduce(acc, acc, K, ReduceOp.add)
                return acc

            # init: z = P^-1 r; w = Mz; v = P^-1 w; p=z, q=w, s=v
            z0_p = precond_psum(r_s, "pcz")
            nc.vector.tensor_copy(z_s, z0_p)
            nc.vector.tensor_copy(p_s, z0_p)
            if debug:
                z0dbg = sp.tile([K, K], F32, tag="dbg_z0", name="dbg_z0")
                nc.vector.tensor_copy(z0dbg, z0_p)
                nc.sync.dma_start(dbg["d_z0"][:, :], z0dbg)
            rz0 = dot_b(r_s, z_s, "rz")
            rzrec = wp.tile([K, 1], F32, tag="rzrec", name="rzrec")
            nc.vector.reciprocal(rzrec, rz0)
            w_s = matvec_z("init")
            nc.vector.tensor_copy(q_s, w_s)
            v_p = precond_psum(w_s, "pcv")
            nc.vector.tensor_copy(s_s, v_p)

            for it in range(N_ITERS):
                # alpha: an = rz/<p,q>_neg = -alpha (dots negated via scale)
                pq = dot_b(p_s, q_s, "pq", scale=-1.0)
                pqr = wp.tile([K, 1], F32, tag="pqr", name="pqr")
                nc.vector.reciprocal(pqr, pq)
                an = wp.tile([K, 1], F32, tag="an", name="an")
                nc.vector.tensor_mul(an, rz0, pqr)
                if it < N_ITERS - 1:
                    nc.vector.scalar_tensor_tensor(
                        r_s, q_s, an, r_s,
                        op0=mybir.AluOpType.mult, op1=mybir.AluOpType.add)
                    nc.vector.scalar_tensor_tensor(
                        z_s, s_s, an, z_s,
                        op0=mybir.AluOpType.mult, op1=mybir.AluOpType.add)
                nc.vector.scalar_tensor_tensor(
                    yn_s, p_s, an, yn_s,
                    op0=mybir.AluOpType.mult, op1=mybir.AluOpType.add)

                if it == N_ITERS - 1:
                    break

                # rz_new, beta; w/v for the NEXT q,s updates
                rz_new = dot_b(r_s, z_s, "rz")
                w_s = matvec_z(f"i{it}")
                if it < N_ITERS - 2:
                    v_p = precond_psum(w_s, "pcv")
                bt = wp.tile([K, 1], F32, tag="bt", name="bt")
                nc.vector.tensor_mul(bt, rz_new, rzrec)
                nc.vector.scalar_tensor_tensor(
                    p_s, p_s, bt, z_s,
                    op0=mybir.AluOpType.mult, op1=mybir.AluOpType.add)
                nc.vector.scalar_tensor_tensor(
                    q_s, q_s, bt, w_s,
                    op0=mybir.AluOpType.mult, op1=mybir.AluOpType.add)
                if it < N_ITERS - 2:
                    nc.vector.scalar_tensor_tensor(
                        s_s, s_s, bt, v_p,
                        op0=mybir.AluOpType.mult, op1=mybir.AluOpType.add)
                rz0 = rz_new
                rzrec = wp.tile([K, 1], F32, tag="rzrec", name="rzrec")
                nc.vector.reciprocal(rzrec, rz0)

            # ------- output: C = Y Mx^T = Yneg (-Mx^T) ----------------------
            yt_p = ps_tile([K, K])
            nc.tensor.transpose(yt_p, yn_s, id64)
            yt_s = wp.tile([K, K], F32, tag="yt_s", name="yt_s")
            nc.vector.tensor_copy(yt_s, yt_p)
            c_p = ps_tile([K, K])
            nc.tensor.matmul(c_p, yt_s, mxTn_s)     # Yneg (-Mx^T)
            c_s = wp.tile([K, K], F32, tag="c_s", name="c_s")
            nc.vector.tensor_copy(c_s, c_p)
            nc.sync.dma_start(out_d[:, :], c_s)

    nc.finalize()
    return nc


def get_program(shard: bool = False, debug: bool = False):
    key = ("prog", debug)
    if key not in _PROGRAM_CACHE:
        _PROGRAM_CACHE[key] = build_program(shard, debug=debug)
    return _PROGRAM_CACHE[key]


def make_in_maps(inputs, shard: bool = False):
    bf = ml_dtypes.bfloat16
    fx = np.ascontiguousarray(np.asarray(inputs["feat_x"], np.float32)[0]).astype(
        bf).reshape(CHUNK, NCH * C)
    fy = np.ascontiguousarray(np.asarray(inputs["feat_y"], np.float32)[0]).astype(
        bf).reshape(CHUNK, NCH * C)
    pxT = np.ascontiguousarray(
        np.asarray(inputs["evecs_trans_x"], np.float32)[0].T).astype(
        bf).reshape(CHUNK, NCH * K)
    pyT = np.ascontiguousarray(
        np.asarray(inputs["evecs_trans_y"], np.float32)[0].T).astype(
        bf).reshape(CHUNK, NCH * K)
    mx = np.ascontiguousarray(np.asarray(inputs["sqrtMk_x"], np.float32)[0])
    my = np.ascontiguousarray(np.asarray(inputs["sqrtMk_y"], np.float32)[0])
    ev = np.ascontiguousarray(np.concatenate([
        np.asarray(inputs["evals_x"], np.float32)[0],
        np.asarray(inputs["evals_y"], np.float32)[0],
    ])[None, :])
    # packed smalls: my | mx | I | -Mx^T   (negated: y is accumulated as -y)
    small = np.ascontiguousarray(np.concatenate(
        [my, mx, np.eye(64, dtype=np.float32), -mx.T], axis=1))
    m = {
        "fx": fx, "fy": fy, "pxT": pxT, "pyT": pyT,
        "small": small, "ev": ev,
    }
    return [dict(m) for _ in range(N_CORES)]


def kernel(**inputs) -> np.ndarray:
    nc = get_program(SHARD)
    in_maps = make_in_maps(inputs, SHARD)
    res = run_bass_kernel_spmd(nc, in_maps, core_ids=list(range(N_CORES)))
    out = np.asarray(res.results[0]["out"], dtype=np.float32)
    return out[None]


# revision 48
# speedup vs baseline: 1.0210x; 1.0210x over previous
"""Trainium2 Bass kernel for nn_ExpandedResolventFMNet.

Mathematical reformulation (validated in fp64 against the jax reference):

The reference builds kron(A.T, My) [8192x4096], its Gram [4096^2], resolvent
kron masks, and solves a dense 4096x4096 system.  All of that collapses:

  first        = kron(A A^T, G),              G = My^T My
  second       = kron-sum of 64x64 factors; the device runs the transposed
                 system in Y = W^T:
  M'(Y)        = G Y S~ + sum_d DdT * (G (DdT * Y)),   C = Y Mx^T
  S~           = Mx^T (A A^T) Mx
  rhs R'       = G By A^T Mx      (uses My^T My By = G By)
  DdT          = resolvent-mask difference matrices (64x64), * = Hadamard

solved by pipelined PCG with the exact-kron preconditioner P^-1 x = Gi x Si
(Gi ~= G^-1, Si ~= S~^-1 via on-device bf16 Newton-Schulz).

Implementation decisions (each backed by a trace or an offline numerics
study; baseline 163.5us -> 83.6us at rel_err 7.4e-3 vs the 2e-2 gate):
 - No collectives: each of the 8 cores runs the full problem redundantly
   and core 0 is read back.  The sharded baseline lost 59us to a 37us
   first-collective barrier + 2x11us AllReduce.
 - Projections in bf16 (input rounding amplifies only ~1.3x through the
   solve; fp32 matmuls are 4 cycles/row vs 1 for bf16).
 - Gram chain and rhs in float32r (single-pass, ~12-bit mantissa measured
   1.25e-4 median rounding on HW); PCG matvec and preconditioner
   applications also f32r: bf16 applications inside the pipelined
   s-recurrence drift to 1.4e-2 (nonlinear rounding compounds in the
   recurrence), f32r holds ~6e-3.
 - gpsimd runs ONLY partition_all_reduce: mixing op families on the Q7
   (elementwise / memset / SWDGE DMA) forces ~7us library reloads.
 - Big inputs are host-pre-arranged to [125, 40*dim] so partition p owns
   vertex rows 40p..40p+39 as one contiguous line; the DMA then coalesces
   to full-line descriptors (a strided rearrange view ran at ~60GB/s).
   6 DMAs total, x-side first on both HWDGE rings (>8 in-flight DMAs
   alias the 8 completion lanes and chain unrelated waits).
 - The y accumulation carries a negated sign (alpha reuses the negated
   <p,q> dot, saving vector ops); the sign is folded into a host-negated
   Mx^T at the output matmul.
 - A warm-up matmul burst during the DMA window ramps the PE p-state.
"""

import numpy as np
import ml_dtypes

import concourse.bacc as bacc
import concourse.mybir as mybir
from concourse.bass_isa import ReduceOp
from concourse.bass_utils import run_bass_kernel_spmd
from concourse.tile import TileContext

F32 = mybir.dt.float32
F32R = mybir.dt.float32r
BF16 = mybir.dt.bfloat16
K = 64          # spectral basis size
C = 128         # feature channels
V = 5000        # vertices
CHUNK = 125     # v-contraction tile (partition dim)
NCH = V // CHUNK                 # 40 chunks
# Single core: the solve is one serial latency-bound chain, and running it
# replicated on all 8 cores makes each core's 3.84MB input pull contend for
# chip HBM (measured ~60GB/s/core vs 358 alone) with zero benefit — only
# core 0's output is read.  One core owns the full ~358GB/s.
N_CORES = 1
N_ITERS = 6
NEWTON_STEPS_S = 6
NEWTON_STEPS_G = 3
SQRT_LMBDA = 10.0

SHARD = False   # kept for test.py compat; ignored (always replicated)

_PROGRAM_CACHE = {}


def build_program(shard: bool = False, debug: bool = False):
    nc = bacc.Bacc("TRN2", num_devices=N_CORES)
    dbg = {}
    if debug:
        for nm, shp in (("d_at", [C, K]), ("d_byt", [C, K]), ("d_st", [K, K]),
                        ("d_gi", [K, K]), ("d_si", [K, K]), ("d_r", [K, K]),
                        ("d_g", [K, K]), ("d_z0", [K, K])):
            dbg[nm] = nc.dram_tensor(nm, shp, F32, kind="ExternalOutput")

    # big inputs are host-pre-arranged to [CHUNK, NCH*dim]: partition p owns
    # vertex rows 40p..40p+39 as one contiguous 10KB/5KB line, so each DMA
    # coalesces into full-line descriptors (the (p n) c rearrange view kept
    # 256B descriptors and ran at ~60GB/s).
    fx_d = nc.dram_tensor("fx", [CHUNK, NCH * C], BF16, kind="ExternalInput")
    fy_d = nc.dram_tensor("fy", [CHUNK, NCH * C], BF16, kind="ExternalInput")
    pxT_d = nc.dram_tensor("pxT", [CHUNK, NCH * K], BF16, kind="ExternalInput")
    pyT_d = nc.dram_tensor("pyT", [CHUNK, NCH * K], BF16, kind="ExternalInput")
    small_d = nc.dram_tensor("small", [K, 4 * K], F32, kind="ExternalInput")
    ev_d = nc.dram_tensor("ev", [1, 2 * K], F32, kind="ExternalInput")
    out_d = nc.dram_tensor("out", [K, K], F32, kind="ExternalOutput")

    with TileContext(nc) as tc:
        with (
            tc.tile_pool(name="big", bufs=1) as bp,
            tc.tile_pool(name="persist", bufs=1) as sp,
            tc.tile_pool(name="work", bufs=2) as wp,
            tc.tile_pool(name="psum", bufs=2, space="PSUM") as pp,
        ):
            # rotating psum tags: 3 tags x bufs=2 -> 6 banks (+2 proj acc = 8)
            _ps_state = {"i": 0}

            def ps_tile(shape):
                i = _ps_state["i"]
                _ps_state["i"] += 1
                return pp.tile(shape, F32, tag=f"ps{i % 3}", name=f"pst{i}")

            # ---------------- input DMA ------------------------------------
            # gpsimd runs ONLY partition_all_reduce in this kernel: every
            # other op family (elementwise, memset, affine_select, SWDGE DMA)
            # forces a Q7 library reload costing ~7us.  Identity comes from
            # the host; y-side DMAs ride the scalar-engine HWDGE ring so x
            # and y stream in parallel.
            # all four [K,K] small tensors ride in ONE packed DMA: every
            # extra DMA costs a semaphore lane and queue slot, and lane
            # aliasing was observed to chain projection matmuls behind
            # unrelated late small DMAs.
            small_s = sp.tile([K, 4 * K], F32)
            ev_t = sp.tile([1, 2 * K], F32)
            nc.sync.dma_start(ev_t, ev_d[:, :])
            nc.sync.dma_start(small_s, small_d[:, :])
            my_s = small_s[:, 0:K]
            mx_s = small_s[:, K:2 * K]
            id64 = small_s[:, 2 * K:3 * K]
            mxTn_s = small_s[:, 3 * K:4 * K]

            # partition p holds vertex rows [p*NCH, (p+1)*NCH) — contiguous
            # 10KB/5KB per partition line -> near-line-rate DMA.  Chunk n of
            # the V-contraction is rows {p*NCH + n}; any partition of V into
            # chunks is valid for the sum.
            # 6 DMAs total: more than 8 in flight aliases the shared HWDGE
            # completion lanes and chains projections behind unrelated DMAs.
            # x-side tensors lead on BOTH rings (the rings share one ~358GB/s
            # HBM pipe, so queue order sets completion order).
            fx_t = bp.tile([CHUNK, NCH * C], BF16)
            fy_t = bp.tile([CHUNK, NCH * C], BF16)
            pxT_t = bp.tile([CHUNK, NCH * K], BF16)
            pyT_t = bp.tile([CHUNK, NCH * K], BF16)
            nc.sync.dma_start(fx_t, fx_d[:, :])
            nc.scalar.dma_start(pxT_t, pxT_d[:, :])
            nc.scalar.dma_start(pyT_t, pyT_d[:, :])
            nc.sync.dma_start(fy_t, fy_d[:, :])

            ones_row = sp.tile([1, K], F32)
            nc.vector.memset(ones_row, 1.0)

            # ------- pre-work that only needs the small tensors -------------
            evmax = sp.tile([1, 1], F32)
            nc.vector.tensor_reduce(evmax, ev_t, mybir.AxisListType.X,
                                    mybir.AluOpType.max)
            evrec = sp.tile([1, 1], F32)
            nc.vector.reciprocal(evrec, evmax)
            t_t = sp.tile([1, 2 * K], F32)
            nc.vector.tensor_scalar_mul(t_t, ev_t, evrec)
            tp1 = sp.tile([1, 2 * K], F32)
            nc.vector.tensor_scalar_add(tp1, t_t, 1.0)
            im_t = sp.tile([1, 2 * K], F32)
            nc.vector.reciprocal(im_t, tp1)
            sq_t = sp.tile([1, 2 * K], F32)
            nc.scalar.sqrt(sq_t, t_t)
            re_t = sp.tile([1, 2 * K], F32)
            nc.vector.tensor_mul(re_t, sq_t, im_t)
            nc.vector.tensor_scalar_mul(re_t, re_t, SQRT_LMBDA)
            nc.vector.tensor_scalar_mul(im_t, im_t, SQRT_LMBDA)

            # D-mask matmuls are emitted later (after S~) so they never sit
            # in front of the projections on the in-order tensor queue; the
            # builder closure runs once its emission point is reached.
            def emit_dmasks():
                d_s = []
                for idx, src in enumerate((re_t, im_t)):
                    pa = ps_tile([K, K])
                    nc.tensor.matmul(pa, src[0:1, K:2 * K], ones_row)
                    pb = ps_tile([K, K])
                    nc.tensor.matmul(pb, ones_row, src[0:1, 0:K])
                    ta = sp.tile([K, K], F32, tag=f"dta{idx}",
                                 name=f"dta{idx}")
                    nc.vector.tensor_copy(ta, pa)
                    dt = sp.tile([K, K], F32, tag=f"d{idx}t_s",
                                 name=f"d{idx}t_s")
                    nc.vector.tensor_sub(dt, ta, pb)
                    d_s.append(dt)
                d12 = sp.tile([K, 2 * K], F32)
                nc.vector.tensor_copy(d12[:, 0:K], d_s[0])
                nc.vector.tensor_copy(d12[:, K:2 * K], d_s[1])
                return d_s[0], d_s[1], d12

            # G = My^T My  (fp32; f32r copy for the solver)
            g_p = ps_tile([K, K])
            nc.tensor.matmul(g_p, my_s, my_s)
            g_s = sp.tile([K, K], F32)
            nc.vector.tensor_copy(g_s, g_p)
            g_bf = sp.tile([K, K], BF16)
            nc.scalar.copy(g_bf, g_p)
            g_r = sp.tile([K, K], F32R)
            nc.vector.tensor_copy(g_r, g_p)
            mx_r = sp.tile([K, K], F32R)
            nc.vector.tensor_copy(mx_r, mx_s)

            # Newton-Schulz inverse in bf16 (preconditioner values only need
            # ~1% accuracy).  X' = 2X - X S X, X0 = I / max_rowsum(|S|).
            _newton_out = {}

            def newton_inverse_bf(mat_bf, mat_f32, tag, steps):
                rs = sp.tile([K, 1], F32, tag=f"{tag}_rs", name=f"{tag}_rs")
                nc.vector.tensor_reduce(rs, mat_f32, mybir.AxisListType.X,
                                        mybir.AluOpType.add,
                                        apply_absolute_value=True)
                nc.gpsimd.partition_all_reduce(rs, rs, K, ReduceOp.max)
                al = sp.tile([K, 1], F32, tag=f"{tag}_al", name=f"{tag}_al")
                nc.vector.reciprocal(al, rs)
                x_bf = sp.tile([K, K], BF16, tag=f"{tag}_x0", name=f"{tag}_x0")
                nc.vector.tensor_scalar_mul(x_bf, id64, al)
                for it in range(steps):
                    t1 = ps_tile([K, K])
                    nc.tensor.matmul(t1, mat_bf, x_bf)        # S X (S sym)
                    t1s = wp.tile([K, K], BF16, tag=f"{tag}_t1s",
                                  name=f"{tag}_t1s")
                    nc.vector.tensor_copy(t1s, t1)
                    t2 = ps_tile([K, K])
                    nc.tensor.matmul(t2, x_bf, t1s)           # X (S X) (X sym)
                    xn = sp.tile([K, K], BF16, tag=f"{tag}_x{it + 1}",
                                 name=f"{tag}_x{it + 1}")
                    nc.vector.scalar_tensor_tensor(
                        xn, x_bf, 2.0, t2,
                        op0=mybir.AluOpType.mult,
                        op1=mybir.AluOpType.subtract)
                    x_bf = xn
                    yield
                # f32r copy for the solver-side applications
                x_r = sp.tile([K, K], F32R, tag=f"{tag}_xr", name=f"{tag}_xr")
                nc.vector.tensor_copy(x_r, x_bf)
                _newton_out[tag] = x_r

            # ---------------- x projection: A^T = fx^T pxT ------------------
            with tc.tile_pool(name="pacc", bufs=1, space="PSUM") as pacc:
                at_p = pacc.tile([C, K], F32)    # A^T  [C,K]
                byt_p = pacc.tile([C, K], F32)   # By^T [C,K]
                for n in range(NCH):
                    nc.tensor.matmul(at_p, fx_t[:, n * C:(n + 1) * C],
                                     pxT_t[:, n * K:(n + 1) * K],
                                     start=(n == 0), stop=(n == NCH - 1))
                at_r = sp.tile([C, K], F32R)
                nc.vector.tensor_copy(at_r, at_p)
                if debug:
                    nc.sync.dma_start(dbg["d_at"][:, :], at_r.bitcast(F32))

                # Gi-Newton sits after the projections in the tensor queue so
                # the gpsimd library load (first partition_all_reduce) never
                # stalls the projection matmuls.
                for _ in newton_inverse_bf(g_bf, g_s, "gi", NEWTON_STEPS_G):
                    pass
                gi_r = _newton_out["gi"]

                # ---- S~ = Mx^T (A A^T) Mx (f32r, symmetric tricks) ----
                sa_p = ps_tile([K, K])
                nc.tensor.matmul(sa_p, at_r, at_r)          # A A^T
                sa_r = sp.tile([K, K], F32R)
                nc.scalar.copy(sa_r, sa_p)
                h1_p = ps_tile([K, K])
                nc.tensor.matmul(h1_p, sa_r, mx_r)          # S_A Mx (sym)
                h1_r = sp.tile([K, K], F32R)
                nc.scalar.copy(h1_r, h1_p)
                st_p = ps_tile([K, K])
                nc.tensor.matmul(st_p, mx_r, h1_r)          # Mx^T S_A Mx
                st_s = sp.tile([K, K], F32)
                nc.vector.tensor_copy(st_s, st_p)
                st_bf = sp.tile([K, K], BF16)
                nc.scalar.copy(st_bf, st_p)
                st_r = sp.tile([K, K], F32R)
                nc.vector.tensor_copy(st_r, st_p)
                if debug:
                    nc.sync.dma_start(dbg["d_st"][:, :], st_s)

                d1t_s, d2t_s, d12t_s = emit_dmasks()

                # ---- Newton-Si (bf16) + eager y projections + rhs chain ---
                ystate = {"n": 0}

                def emit_y(k):
                    for _ in range(k):
                        n = ystate["n"]
                        if n >= NCH:
                            return
                        ystate["n"] += 1
                        nc.tensor.matmul(byt_p, fy_t[:, n * C:(n + 1) * C],
                                         pyT_t[:, n * K:(n + 1) * K],
                                         start=(n == 0), stop=(n == NCH - 1))

                rhs_out = {}

                def emit_rhs():
                    """rhs R' = G By A^T Mx (f32r) — independent of Si."""
                    byt_r = sp.tile([C, K], F32R)
                    nc.vector.tensor_copy(byt_r, byt_p)
                    if debug:
                        nc.sync.dma_start(dbg["d_byt"][:, :],
                                          byt_r.bitcast(F32))
                    byat_p = ps_tile([K, K])
                    nc.tensor.matmul(byat_p, byt_r, at_r)   # By A^T
                    byat_r = sp.tile([K, K], F32R)
                    nc.scalar.copy(byat_r, byat_p)
                    w0_p = ps_tile([K, K])
                    nc.tensor.matmul(w0_p, g_r, byat_r)     # G By A^T (G sym)
                    w0_s = sp.tile([K, K], F32)
                    nc.vector.tensor_copy(w0_s, w0_p)
                    wt_p = ps_tile([K, K])
                    nc.tensor.transpose(wt_p, w0_s, id64)
                    wt_r = sp.tile([K, K], F32R)
                    nc.scalar.copy(wt_r, wt_p)
                    rp_p = ps_tile([K, K])
                    nc.tensor.matmul(rp_p, wt_r, mx_r)      # (G By A^T) Mx
                    r_t = sp.tile([K, K], F32)              # CG residual
                    nc.vector.tensor_copy(r_t, rp_p)
                    rhs_out["r_s"] = r_t

                emit_y(4)
                for _ in newton_inverse_bf(st_bf, st_s, "si", NEWTON_STEPS_S):
                    emit_y(9)
                    if ystate["n"] >= NCH and "r_s" not in rhs_out:
                        emit_rhs()
                si_r = _newton_out["si"]
                emit_y(NCH)
                if "r_s" not in rhs_out:
                    emit_rhs()
                r_s = rhs_out["r_s"]

            if debug:
                nc.sync.dma_start(dbg["d_r"][:, :], r_s)
                gi_f = sp.tile([K, K], F32, tag="dbg_gi", name="dbg_gi")
                nc.vector.tensor_copy(gi_f, gi_r)
                nc.sync.dma_start(dbg["d_gi"][:, :], gi_f)
                si_f = sp.tile([K, K], F32, tag="dbg_si", name="dbg_si")
                nc.vector.tensor_copy(si_f, si_r)
                nc.sync.dma_start(dbg["d_si"][:, :], si_f)
                nc.sync.dma_start(dbg["d_g"][:, :], g_s)

            # ------- pipelined PCG (f32r matvec + precond applications) -----
            yn_s = sp.tile([K, K], F32)      # accumulates -y
            nc.vector.memset(yn_s, 0.0)
            p_s = sp.tile([K, K], F32)
            q_s = sp.tile([K, K], F32)
            s_s = sp.tile([K, K], F32)
            z_s = sp.tile([K, K], F32R)      # f32r so matvec mms read it
            u_s = sp.tile([K, 2 * K], F32R)  # stacked [D1T*z | D2T*z]

            def precond_psum(x_tile, tag):
                """P^-1 x in PSUM: (Gi x)^T = mm(lhsT=x_r, Gi); then * Si."""
                xb = wp.tile([K, K], F32R, tag=f"{tag}_xb", name=f"{tag}_xb")
                nc.scalar.copy(xb, x_tile)
                ut_p = ps_tile([K, K])
                nc.tensor.matmul(ut_p, xb, gi_r)
                ut_s = wp.tile([K, K], F32R, tag=f"{tag}_uts",
                               name=f"{tag}_uts")
                nc.scalar.copy(ut_s, ut_p)
                v_p = ps_tile([K, K])
                nc.tensor.matmul(v_p, ut_s, si_r)
                return v_p

            def matvec_z(tag):
                """w = M z into SBUF (reads z_s); f32r matmuls."""
                nc.vector.tensor_mul(u_s[:, 0:K], d1t_s, z_s)
                nc.vector.tensor_mul(u_s[:, K:2 * K], d2t_s, z_s)
                gzt_p = ps_tile([K, K])
                nc.tensor.matmul(gzt_p, z_s, g_r)         # (G z)^T
                gzt_s = wp.tile([K, K], F32R, tag="mv_gzts", name="mv_gzts")
                nc.vector.tensor_copy(gzt_s, gzt_p)
                t2_p = ps_tile([K, K])
                nc.tensor.matmul(t2_p, gzt_s, st_r)       # (G z) S~
                gu_p = ps_tile([K, 2 * K])
                nc.tensor.matmul(gu_p, g_r, u_s)          # G [u1|u2], one mm
                mm_s = wp.tile([K, 2 * K], F32, tag="mv_mm", name="mv_mm")
                nc.vector.tensor_mul(mm_s, d12t_s, gu_p)  # masked, both halves
                a1_s = wp.tile([K, K], F32, tag="mv_a1", name="mv_a1")
                nc.vector.tensor_add(a1_s, mm_s[:, 0:K], t2_p)
                w_s = wp.tile([K, K], F32, tag="mv_w", name="mv_w")
                nc.vector.tensor_add(w_s, a1_s, mm_s[:, K:2 * K])
                return w_s

            def dot_b(a_ap, b_ap, tag, scale=1.0):
                """<a,b>*scale broadcast to all partitions as [K,1] SBUF."""
                prod = wp.tile([K, K], F32, tag="dot_dm", name="dot_dm")
                acc = wp.tile([K, 1], F32, tag=f"{tag}_acc", name=f"{tag}_acc")
                nc.vector.scalar_tensor_tensor(
                    prod, a_ap, scale, b_ap,
                    op0=mybir.AluOpType.mult, op1=mybir.AluOpType.mult,
                    accum_out=acc)
                nc.gpsimd.partition_all_reduce(acc, acc, K, ReduceOp.add)
                return acc

            # init: z = P^-1 r; w = Mz; v = P^-1 w; p=z, q=w, s=v
            z0_p = precond_psum(r_s, "pcz")
            nc.vector.tensor_copy(z_s, z0_p)
            nc.vector.tensor_copy(p_s, z0_p)
            if debug:
                z0dbg = sp.tile([K, K], F32, tag="dbg_z0", name="dbg_z0")
                nc.vector.tensor_copy(z0dbg, z0_p)
                nc.sync.dma_start(dbg["d_z0"][:, :], z0dbg)
            rz0 = dot_b(r_s, z_s, "rz")
            rzrec = wp.tile([K, 1], F32, tag="rzrec", name="rzrec")
            nc.vector.reciprocal(rzrec, rz0)
            w_s = matvec_z("init")
            nc.vector.tensor_copy(q_s, w_s)
            v_p = precond_psum(w_s, "pcv")
            nc.vector.tensor_copy(s_s, v_p)

            for it in range(N_ITERS):
                # alpha: an = rz/<p,q>_neg = -alpha (dots negated via scale)
                pq = dot_b(p_s, q_s, "pq", scale=-1.0)
                pqr = wp.tile([K, 1], F32, tag="pqr", name="pqr")
                nc.vector.reciprocal(pqr, pq)
                an = wp.tile([K, 1], F32, tag="an", name="an")
                nc.vector.tensor_mul(an, rz0, pqr)
                if it < N_ITERS - 1:
                    nc.vector.scalar_tensor_tensor(
                        r_s, q_s, an, r_s,
                        op0=mybir.AluOpType.mult, op1=mybir.AluOpType.add)
                    nc.vector.scalar_tensor_tensor(
                        z_s, s_s, an, z_s,
                        op0=mybir.AluOpType.mult, op1=mybir.AluOpType.add)
                nc.vector.scalar_tensor_tensor(
                    yn_s, p_s, an, yn_s,
                    op0=mybir.AluOpType.mult, op1=mybir.AluOpType.add)

                if it == N_ITERS - 1:
                    break

                # rz_new, beta; w/v for the NEXT q,s updates
                rz_new = dot_b(r_s, z_s, "rz")
                w_s = matvec_z(f"i{it}")
                if it < N_ITERS - 2:
                    v_p = precond_psum(w_s, "pcv")
                bt = wp.tile([K, 1], F32, tag="bt", name="bt")
                nc.vector.tensor_mul(bt, rz_new, rzrec)
                nc.vector.scalar_tensor_tensor(
                    p_s, p_s, bt, z_s,
                    op0=mybir.AluOpType.mult, op1=mybir.AluOpType.add)
                nc.vector.scalar_tensor_tensor(
                    q_s, q_s, bt, w_s,
                    op0=mybir.AluOpType.mult, op1=mybir.AluOpType.add)
                if it < N_ITERS - 2:
                    nc.vector.scalar_tensor_tensor(
                        s_s, s_s, bt, v_p,
                        op0=mybir.AluOpType.mult, op1=mybir.AluOpType.add)
                rz0 = rz_new
                rzrec = wp.tile([K, 1], F32, tag="rzrec", name="rzrec")
                nc.vector.reciprocal(rzrec, rz0)

            # ------- output: C = Y Mx^T = Yneg (-Mx^T) ----------------------
            yt_p = ps_tile([K, K])
            nc.tensor.transpose(yt_p, yn_s, id64)
            yt_s = wp.tile([K, K], F32, tag="yt_s", name="yt_s")
            nc.vector.tensor_copy(yt_s, yt_p)
            c_p = ps_tile([K, K])
            nc.tensor.matmul(c_p, yt_s, mxTn_s)     # Yneg (-Mx^T)
            c_s = wp.tile([K, K], F32, tag="c_s", name="c_s")
            nc.vector.tensor_copy(c_s, c_p)
            nc.sync.dma_start(out_d[:, :], c_s)

    nc.finalize()
    return nc


def get_program(shard: bool = False, debug: bool = False):
    key = ("prog", debug)
    if key not in _PROGRAM_CACHE:
        _PROGRAM_CACHE[key] = build_program(shard, debug=debug)
    return _PROGRAM_CACHE[key]


def make_in_maps(inputs, shard: bool = False):
    bf = ml_dtypes.bfloat16
    fx = np.ascontiguousarray(np.asarray(inputs["feat_x"], np.float32)[0]).astype(
        bf).reshape(CHUNK, NCH * C)
    fy = np.ascontiguousarray(np.asarray(inputs["feat_y"], np.float32)[0]).astype(
        bf).reshape(CHUNK, NCH * C)
    pxT = np.ascontiguousarray(
        np.asarray(inputs["evecs_trans_x"], np.float32)[0].T).astype(
        bf).reshape(CHUNK, NCH * K)
    pyT = np.ascontiguousarray(
        np.asarray(inputs["evecs_trans_y"], np.float32)[0].T).astype(
        bf).reshape(CHUNK, NCH * K)
    mx = np.ascontiguousarray(np.asarray(inputs["sqrtMk_x"], np.float32)[0])
    my = np.ascontiguousarray(np.asarray(inputs["sqrtMk_y"], np.float32)[0])
    ev = np.ascontiguousarray(np.concatenate([
        np.asarray(inputs["evals_x"], np.float32)[0],
        np.asarray(inputs["evals_y"], np.float32)[0],
    ])[None, :])
    # packed smalls: my | mx | I | -Mx^T   (negated: y is accumulated as -y)
    small = np.ascontiguousarray(np.concatenate(
        [my, mx, np.eye(64, dtype=np.float32), -mx.T], axis=1))
    m = {
        "fx": fx, "fy": fy, "pxT": pxT, "pyT": pyT,
        "small": small, "ev": ev,
    }
    return [dict(m) for _ in range(N_CORES)]


def kernel(**inputs) -> np.ndarray:
    nc = get_program(SHARD)
    in_maps = make_in_maps(inputs, SHARD)
    res = run_bass_kernel_spmd(nc, in_maps, core_ids=list(range(N_CORES)))
    out = np.asarray(res.results[0]["out"], dtype=np.float32)
    return out[None]


# revision 50
# speedup vs baseline: 1.0211x; 1.0001x over previous
"""Trainium2 Bass kernel for nn_ExpandedResolventFMNet.

Mathematical reformulation (validated in fp64 against the jax reference):

The reference builds kron(A.T, My) [8192x4096], its Gram [4096^2], resolvent
kron masks, and solves a dense 4096x4096 system.  All of that collapses:

  first        = kron(A A^T, G),              G = My^T My
  second       = kron-sum of 64x64 factors; the device runs the transposed
                 system in Y = W^T:
  M'(Y)        = G Y S~ + sum_d DdT * (G (DdT * Y)),   C = Y Mx^T
  S~           = Mx^T (A A^T) Mx
  rhs R'       = G By A^T Mx      (uses My^T My By = G By)
  DdT          = resolvent-mask difference matrices (64x64), * = Hadamard

solved by pipelined PCG with the exact-kron preconditioner P^-1 x = Gi x Si
(Gi ~= G^-1, Si ~= S~^-1 via on-device bf16 Newton-Schulz).

Implementation decisions (each backed by a trace or an offline numerics
study; baseline 163.5us -> 83.6us at rel_err 7.4e-3 vs the 2e-2 gate):
 - No collectives: each of the 8 cores runs the full problem redundantly
   and core 0 is read back.  The sharded baseline lost 59us to a 37us
   first-collective barrier + 2x11us AllReduce.
 - Projections in bf16 (input rounding amplifies only ~1.3x through the
   solve; fp32 matmuls are 4 cycles/row vs 1 for bf16).
 - Gram chain and rhs in float32r (single-pass, ~12-bit mantissa measured
   1.25e-4 median rounding on HW); PCG matvec and preconditioner
   applications also f32r: bf16 applications inside the pipelined
   s-recurrence drift to 1.4e-2 (nonlinear rounding compounds in the
   recurrence), f32r holds ~6e-3.
 - gpsimd runs ONLY partition_all_reduce: mixing op families on the Q7
   (elementwise / memset / SWDGE DMA) forces ~7us library reloads.
 - Big inputs are host-pre-arranged to [125, 40*dim] so partition p owns
   vertex rows 40p..40p+39 as one contiguous line; the DMA then coalesces
   to full-line descriptors (a strided rearrange view ran at ~60GB/s).
   6 DMAs total, x-side first on both HWDGE rings (>8 in-flight DMAs
   alias the 8 completion lanes and chain unrelated waits).
 - The y accumulation carries a negated sign (alpha reuses the negated
   <p,q> dot, saving vector ops); the sign is folded into a host-negated
   Mx^T at the output matmul.
"""

import numpy as np
import ml_dtypes

import concourse.bacc as bacc
import concourse.mybir as mybir
from concourse.bass_isa import ReduceOp
from concourse.bass_utils import run_bass_kernel_spmd
from concourse.tile import TileContext

F32 = mybir.dt.float32
F32R = mybir.dt.float32r
BF16 = mybir.dt.bfloat16
K = 64          # spectral basis size
C = 128         # feature channels
V = 5000        # vertices
CHUNK = 125     # v-contraction tile (partition dim)
NCH = V // CHUNK                 # 40 chunks
# Single core: the solve is one serial latency-bound chain, and running it
# replicated on all 8 cores makes each core's 3.84MB input pull contend for
# chip HBM (measured ~60GB/s/core vs 358 alone) with zero benefit — only
# core 0's output is read.  One core owns the full ~358GB/s.
N_CORES = 1
N_ITERS = 6
NEWTON_STEPS_S = 6
NEWTON_STEPS_G = 3
SQRT_LMBDA = 10.0

SHARD = False   # kept for test.py compat; ignored (always replicated)

_PROGRAM_CACHE = {}


def build_program(shard: bool = False, debug: bool = False):
    nc = bacc.Bacc("TRN2", num_devices=N_CORES)
    dbg = {}
    if debug:
        for nm, shp in (("d_at", [C, K]), ("d_byt", [C, K]), ("d_st", [K, K]),
                        ("d_gi", [K, K]), ("d_si", [K, K]), ("d_r", [K, K]),
                        ("d_g", [K, K]), ("d_z0", [K, K])):
            dbg[nm] = nc.dram_tensor(nm, shp, F32, kind="ExternalOutput")

    # big inputs are host-pre-arranged to [CHUNK, NCH*dim]: partition p owns
    # vertex rows 40p..40p+39 as one contiguous 10KB/5KB line, so each DMA
    # coalesces into full-line descriptors (the (p n) c rearrange view kept
    # 256B descriptors and ran at ~60GB/s).
    fx_d = nc.dram_tensor("fx", [CHUNK, NCH * C], BF16, kind="ExternalInput")
    fy_d = nc.dram_tensor("fy", [CHUNK, NCH * C], BF16, kind="ExternalInput")
    pxT_d = nc.dram_tensor("pxT", [CHUNK, NCH * K], BF16, kind="ExternalInput")
    pyT_d = nc.dram_tensor("pyT", [CHUNK, NCH * K], BF16, kind="ExternalInput")
    small_d = nc.dram_tensor("small", [K, 4 * K], F32, kind="ExternalInput")
    ev_d = nc.dram_tensor("ev", [1, 2 * K], F32, kind="ExternalInput")
    out_d = nc.dram_tensor("out", [K, K], F32, kind="ExternalOutput")

    with TileContext(nc) as tc:
        with (
            tc.tile_pool(name="big", bufs=1) as bp,
            tc.tile_pool(name="persist", bufs=1) as sp,
            tc.tile_pool(name="work", bufs=3) as wp,
            tc.tile_pool(name="psum", bufs=2, space="PSUM") as pp,
        ):
            # rotating psum tags: 3 tags x bufs=2 -> 6 banks (+2 proj acc = 8)
            _ps_state = {"i": 0}

            def ps_tile(shape):
                i = _ps_state["i"]
                _ps_state["i"] += 1
                return pp.tile(shape, F32, tag=f"ps{i % 3}", name=f"pst{i}")

            # ---------------- input DMA ------------------------------------
            # gpsimd runs ONLY partition_all_reduce in this kernel: every
            # other op family (elementwise, memset, affine_select, SWDGE DMA)
            # forces a Q7 library reload costing ~7us.  Identity comes from
            # the host; y-side DMAs ride the scalar-engine HWDGE ring so x
            # and y stream in parallel.
            # all four [K,K] small tensors ride in ONE packed DMA: every
            # extra DMA costs a semaphore lane and queue slot, and lane
            # aliasing was observed to chain projection matmuls behind
            # unrelated late small DMAs.
            small_s = sp.tile([K, 4 * K], F32)
            ev_t = sp.tile([1, 2 * K], F32)
            nc.sync.dma_start(ev_t, ev_d[:, :])
            nc.sync.dma_start(small_s, small_d[:, :])
            my_s = small_s[:, 0:K]
            mx_s = small_s[:, K:2 * K]
            id64 = small_s[:, 2 * K:3 * K]
            mxTn_s = small_s[:, 3 * K:4 * K]

            # partition p holds vertex rows [p*NCH, (p+1)*NCH) — contiguous
            # 10KB/5KB per partition line -> near-line-rate DMA.  Chunk n of
            # the V-contraction is rows {p*NCH + n}; any partition of V into
            # chunks is valid for the sum.
            # 6 DMAs total: more than 8 in flight aliases the shared HWDGE
            # completion lanes and chains projections behind unrelated DMAs.
            # x-side tensors lead on BOTH rings (the rings share one ~358GB/s
            # HBM pipe, so queue order sets completion order).
            fx_t = bp.tile([CHUNK, NCH * C], BF16)
            fy_t = bp.tile([CHUNK, NCH * C], BF16)
            pxT_t = bp.tile([CHUNK, NCH * K], BF16)
            pyT_t = bp.tile([CHUNK, NCH * K], BF16)
            nc.sync.dma_start(fx_t, fx_d[:, :])
            nc.scalar.dma_start(pxT_t, pxT_d[:, :])
            nc.scalar.dma_start(pyT_t, pyT_d[:, :])
            nc.sync.dma_start(fy_t, fy_d[:, :])

            ones_row = sp.tile([1, K], F32)
            nc.vector.memset(ones_row, 1.0)

            # ------- pre-work that only needs the small tensors -------------
            evmax = sp.tile([1, 1], F32)
            nc.vector.tensor_reduce(evmax, ev_t, mybir.AxisListType.X,
                                    mybir.AluOpType.max)
            evrec = sp.tile([1, 1], F32)
            nc.vector.reciprocal(evrec, evmax)
            t_t = sp.tile([1, 2 * K], F32)
            nc.vector.tensor_scalar_mul(t_t, ev_t, evrec)
            tp1 = sp.tile([1, 2 * K], F32)
            nc.vector.tensor_scalar_add(tp1, t_t, 1.0)
            im_t = sp.tile([1, 2 * K], F32)
            nc.vector.reciprocal(im_t, tp1)
            sq_t = sp.tile([1, 2 * K], F32)
            nc.scalar.sqrt(sq_t, t_t)
            re_t = sp.tile([1, 2 * K], F32)
            nc.vector.tensor_mul(re_t, sq_t, im_t)
            nc.vector.tensor_scalar_mul(re_t, re_t, SQRT_LMBDA)
            nc.vector.tensor_scalar_mul(im_t, im_t, SQRT_LMBDA)

            # D-mask matmuls are emitted later (after S~) so they never sit
            # in front of the projections on the in-order tensor queue; the
            # builder closure runs once its emission point is reached.
            def emit_dmasks():
                d_s = []
                for idx, src in enumerate((re_t, im_t)):
                    pa = ps_tile([K, K])
                    nc.tensor.matmul(pa, src[0:1, K:2 * K], ones_row)
                    pb = ps_tile([K, K])
                    nc.tensor.matmul(pb, ones_row, src[0:1, 0:K])
                    ta = sp.tile([K, K], F32, tag=f"dta{idx}",
                                 name=f"dta{idx}")
                    nc.vector.tensor_copy(ta, pa)
                    dt = sp.tile([K, K], F32, tag=f"d{idx}t_s",
                                 name=f"d{idx}t_s")
                    nc.vector.tensor_sub(dt, ta, pb)
                    d_s.append(dt)
                d12 = sp.tile([K, 2 * K], F32)
                nc.vector.tensor_copy(d12[:, 0:K], d_s[0])
                nc.vector.tensor_copy(d12[:, K:2 * K], d_s[1])
                return d_s[0], d_s[1], d12

            # G = My^T My  (fp32; f32r copy for the solver)
            g_p = ps_tile([K, K])
            nc.tensor.matmul(g_p, my_s, my_s)
            g_s = sp.tile([K, K], F32)
            nc.vector.tensor_copy(g_s, g_p)
            g_bf = sp.tile([K, K], BF16)
            nc.scalar.copy(g_bf, g_p)
            g_r = sp.tile([K, K], F32R)
            nc.vector.tensor_copy(g_r, g_p)
            mx_r = sp.tile([K, K], F32R)
            nc.vector.tensor_copy(mx_r, mx_s)

            # Newton-Schulz inverse in bf16 (preconditioner values only need
            # ~1% accuracy).  X' = 2X - X S X, X0 = I / max_rowsum(|S|).
            _newton_out = {}

            def newton_inverse_bf(mat_bf, mat_f32, tag, steps):
                rs = sp.tile([K, 1], F32, tag=f"{tag}_rs", name=f"{tag}_rs")
                nc.vector.tensor_reduce(rs, mat_f32, mybir.AxisListType.X,
                                        mybir.AluOpType.add,
                                        apply_absolute_value=True)
                nc.gpsimd.partition_all_reduce(rs, rs, K, ReduceOp.max)
                al = sp.tile([K, 1], F32, tag=f"{tag}_al", name=f"{tag}_al")
                nc.vector.reciprocal(al, rs)
                x_bf = sp.tile([K, K], BF16, tag=f"{tag}_x0", name=f"{tag}_x0")
                nc.vector.tensor_scalar_mul(x_bf, id64, al)
                for it in range(steps):
                    t1 = ps_tile([K, K])
                    nc.tensor.matmul(t1, mat_bf, x_bf)        # S X (S sym)
                    t1s = wp.tile([K, K], BF16, tag=f"{tag}_t1s",
                                  name=f"{tag}_t1s")
                    nc.vector.tensor_copy(t1s, t1)
                    t2 = ps_tile([K, K])
                    nc.tensor.matmul(t2, x_bf, t1s)           # X (S X) (X sym)
                    xn = sp.tile([K, K], BF16, tag=f"{tag}_x{it + 1}",
                                 name=f"{tag}_x{it + 1}")
                    nc.vector.scalar_tensor_tensor(
                        xn, x_bf, 2.0, t2,
                        op0=mybir.AluOpType.mult,
                        op1=mybir.AluOpType.subtract)
                    x_bf = xn
                    yield
                # f32r copy for the solver-side applications
                x_r = sp.tile([K, K], F32R, tag=f"{tag}_xr", name=f"{tag}_xr")
                nc.vector.tensor_copy(x_r, x_bf)
                _newton_out[tag] = x_r

            # ---------------- x projection: A^T = fx^T pxT ------------------
            with tc.tile_pool(name="pacc", bufs=1, space="PSUM") as pacc:
                at_p = pacc.tile([C, K], F32)    # A^T  [C,K]
                byt_p = pacc.tile([C, K], F32)   # By^T [C,K]
                for n in range(NCH):
                    nc.tensor.matmul(at_p, fx_t[:, n * C:(n + 1) * C],
                                     pxT_t[:, n * K:(n + 1) * K],
                                     start=(n == 0), stop=(n == NCH - 1))
                at_r = sp.tile([C, K], F32R)
                nc.vector.tensor_copy(at_r, at_p)
                if debug:
                    nc.sync.dma_start(dbg["d_at"][:, :], at_r.bitcast(F32))

                # Gi-Newton sits after the projections in the tensor queue so
                # the gpsimd library load (first partition_all_reduce) never
                # stalls the projection matmuls.
                for _ in newton_inverse_bf(g_bf, g_s, "gi", NEWTON_STEPS_G):
                    pass
                gi_r = _newton_out["gi"]

                # ---- S~ = Mx^T (A A^T) Mx (f32r, symmetric tricks) ----
                sa_p = ps_tile([K, K])
                nc.tensor.matmul(sa_p, at_r, at_r)          # A A^T
                sa_r = sp.tile([K, K], F32R)
                nc.scalar.copy(sa_r, sa_p)
                h1_p = ps_tile([K, K])
                nc.tensor.matmul(h1_p, sa_r, mx_r)          # S_A Mx (sym)
                h1_r = sp.tile([K, K], F32R)
                nc.scalar.copy(h1_r, h1_p)
                st_p = ps_tile([K, K])
                nc.tensor.matmul(st_p, mx_r, h1_r)          # Mx^T S_A Mx
                st_s = sp.tile([K, K], F32)
                nc.vector.tensor_copy(st_s, st_p)
                st_bf = sp.tile([K, K], BF16)
                nc.scalar.copy(st_bf, st_p)
                st_r = sp.tile([K, K], F32R)
                nc.vector.tensor_copy(st_r, st_p)
                if debug:
                    nc.sync.dma_start(dbg["d_st"][:, :], st_s)

                d1t_s, d2t_s, d12t_s = emit_dmasks()

                # ---- Newton-Si (bf16) + eager y projections + rhs chain ---
                ystate = {"n": 0}

                def emit_y(k):
                    for _ in range(k):
                        n = ystate["n"]
                        if n >= NCH:
                            return
                        ystate["n"] += 1
                        nc.tensor.matmul(byt_p, fy_t[:, n * C:(n + 1) * C],
                                         pyT_t[:, n * K:(n + 1) * K],
                                         start=(n == 0), stop=(n == NCH - 1))

                rhs_out = {}

                def emit_rhs():
                    """rhs R' = G By A^T Mx (f32r) — independent of Si."""
                    byt_r = sp.tile([C, K], F32R)
                    nc.vector.tensor_copy(byt_r, byt_p)
                    if debug:
                        nc.sync.dma_start(dbg["d_byt"][:, :],
                                          byt_r.bitcast(F32))
                    byat_p = ps_tile([K, K])
                    nc.tensor.matmul(byat_p, byt_r, at_r)   # By A^T
                    byat_r = sp.tile([K, K], F32R)
                    nc.scalar.copy(byat_r, byat_p)
                    w0_p = ps_tile([K, K])
                    nc.tensor.matmul(w0_p, g_r, byat_r)     # G By A^T (G sym)
                    w0_s = sp.tile([K, K], F32)
                    nc.vector.tensor_copy(w0_s, w0_p)
                    wt_p = ps_tile([K, K])
                    nc.tensor.transpose(wt_p, w0_s, id64)
                    wt_r = sp.tile([K, K], F32R)
                    nc.scalar.copy(wt_r, wt_p)
                    rp_p = ps_tile([K, K])
                    nc.tensor.matmul(rp_p, wt_r, mx_r)      # (G By A^T) Mx
                    r_t = sp.tile([K, K], F32)              # CG residual
                    nc.vector.tensor_copy(r_t, rp_p)
                    rhs_out["r_s"] = r_t

                emit_y(4)
                for _ in newton_inverse_bf(st_bf, st_s, "si", NEWTON_STEPS_S):
                    emit_y(9)
                    if ystate["n"] >= NCH and "r_s" not in rhs_out:
                        emit_rhs()
                si_r = _newton_out["si"]
                emit_y(NCH)
                if "r_s" not in rhs_out:
                    emit_rhs()
                r_s = rhs_out["r_s"]

            if debug:
                nc.sync.dma_start(dbg["d_r"][:, :], r_s)
                gi_f = sp.tile([K, K], F32, tag="dbg_gi", name="dbg_gi")
                nc.vector.tensor_copy(gi_f, gi_r)
                nc.sync.dma_start(dbg["d_gi"][:, :], gi_f)
                si_f = sp.tile([K, K], F32, tag="dbg_si", name="dbg_si")
                nc.vector.tensor_copy(si_f, si_r)
                nc.sync.dma_start(dbg["d_si"][:, :], si_f)
                nc.sync.dma_start(dbg["d_g"][:, :], g_s)

            # ------- pipelined PCG (f32r matvec + precond applications) -----
            yn_s = sp.tile([K, K], F32)      # accumulates -y
            nc.vector.memset(yn_s, 0.0)
            p_s = sp.tile([K, K], F32)
            q_s = sp.tile([K, K], F32)
            s_s = sp.tile([K, K], F32)
            z_s = sp.tile([K, K], F32R)      # f32r so matvec mms read it
            u_s = sp.tile([K, 2 * K], F32R)  # stacked [D1T*z | D2T*z]

            def precond_psum(x_tile, tag):
                """P^-1 x in PSUM: (Gi x)^T = mm(lhsT=x_r, Gi); then * Si."""
                xb = wp.tile([K, K], F32R, tag=f"{tag}_xb", name=f"{tag}_xb")
                nc.scalar.copy(xb, x_tile)
                ut_p = ps_tile([K, K])
                nc.tensor.matmul(ut_p, xb, gi_r)
                ut_s = wp.tile([K, K], F32R, tag=f"{tag}_uts",
                               name=f"{tag}_uts")
                nc.scalar.copy(ut_s, ut_p)
                v_p = ps_tile([K, K])
                nc.tensor.matmul(v_p, ut_s, si_r)
                return v_p

            def matvec_z(tag):
                """w = M z into SBUF (reads z_s); f32r matmuls."""
                nc.vector.tensor_mul(u_s[:, 0:K], d1t_s, z_s)
                nc.vector.tensor_mul(u_s[:, K:2 * K], d2t_s, z_s)
                gzt_p = ps_tile([K, K])
                nc.tensor.matmul(gzt_p, z_s, g_r)         # (G z)^T
                gzt_s = wp.tile([K, K], F32R, tag="mv_gzts", name="mv_gzts")
                nc.vector.tensor_copy(gzt_s, gzt_p)
                t2_p = ps_tile([K, K])
                nc.tensor.matmul(t2_p, gzt_s, st_r)       # (G z) S~
                gu_p = ps_tile([K, 2 * K])
                nc.tensor.matmul(gu_p, g_r, u_s)          # G [u1|u2], one mm
                mm_s = wp.tile([K, 2 * K], F32, tag="mv_mm", name="mv_mm")
                nc.vector.tensor_mul(mm_s, d12t_s, gu_p)  # masked, both halves
                a1_s = wp.tile([K, K], F32, tag="mv_a1", name="mv_a1")
                nc.vector.tensor_add(a1_s, mm_s[:, 0:K], t2_p)
                w_s = wp.tile([K, K], F32, tag="mv_w", name="mv_w")
                nc.vector.tensor_add(w_s, a1_s, mm_s[:, K:2 * K])
                return w_s

            def dot_b(a_ap, b_ap, tag, scale=1.0):
                """<a,b>*scale broadcast to all partitions as [K,1] SBUF."""
                prod = wp.tile([K, K], F32, tag="dot_dm", name="dot_dm")
                acc = wp.tile([K, 1], F32, tag=f"{tag}_acc", name=f"{tag}_acc")
                nc.vector.scalar_tensor_tensor(
                    prod, a_ap, scale, b_ap,
                    op0=mybir.AluOpType.mult, op1=mybir.AluOpType.mult,
                    accum_out=acc)
                nc.gpsimd.partition_all_reduce(acc, acc, K, ReduceOp.add)
                return acc

            # init: z = P^-1 r; w = Mz; v = P^-1 w; p=z, q=w, s=v
            z0_p = precond_psum(r_s, "pcz")
            nc.vector.tensor_copy(z_s, z0_p)
            nc.vector.tensor_copy(p_s, z0_p)
            if debug:
                z0dbg = sp.tile([K, K], F32, tag="dbg_z0", name="dbg_z0")
                nc.vector.tensor_copy(z0dbg, z0_p)
                nc.sync.dma_start(dbg["d_z0"][:, :], z0dbg)
            rz0 = dot_b(r_s, z_s, "rz")
            rzrec = wp.tile([K, 1], F32, tag="rzrec", name="rzrec")
            nc.vector.reciprocal(rzrec, rz0)
            w_s = matvec_z("init")
            nc.vector.tensor_copy(q_s, w_s)
            v_p = precond_psum(w_s, "pcv")
            nc.vector.tensor_copy(s_s, v_p)

            for it in range(N_ITERS):
                # alpha: an = rz/<p,q>_neg = -alpha (dots negated via scale)
                pq = dot_b(p_s, q_s, "pq", scale=-1.0)
                pqr = wp.tile([K, 1], F32, tag="pqr", name="pqr")
                nc.vector.reciprocal(pqr, pq)
                an = wp.tile([K, 1], F32, tag="an", name="an")
                nc.vector.tensor_mul(an, rz0, pqr)
                if it < N_ITERS - 1:
                    nc.vector.scalar_tensor_tensor(
                        r_s, q_s, an, r_s,
                        op0=mybir.AluOpType.mult, op1=mybir.AluOpType.add)
                    nc.vector.scalar_tensor_tensor(
                        z_s, s_s, an, z_s,
                        op0=mybir.AluOpType.mult, op1=mybir.AluOpType.add)
                nc.vector.scalar_tensor_tensor(
                    yn_s, p_s, an, yn_s,
                    op0=mybir.AluOpType.mult, op1=mybir.AluOpType.add)

                if it == N_ITERS - 1:
                    break

                # rz_new, beta; w/v for the NEXT q,s updates
                rz_new = dot_b(r_s, z_s, "rz")
                w_s = matvec_z(f"i{it}")
                if it < N_ITERS - 2:
                    v_p = precond_psum(w_s, "pcv")
                bt = wp.tile([K, 1], F32, tag="bt", name="bt")
                nc.vector.tensor_mul(bt, rz_new, rzrec)
                nc.vector.scalar_tensor_tensor(
                    p_s, p_s, bt, z_s,
                    op0=mybir.AluOpType.mult, op1=mybir.AluOpType.add)
                nc.vector.scalar_tensor_tensor(
                    q_s, q_s, bt, w_s,
                    op0=mybir.AluOpType.mult, op1=mybir.AluOpType.add)
                if it < N_ITERS - 2:
                    nc.vector.scalar_tensor_tensor(
                        s_s, s_s, bt, v_p,
                        op0=mybir.AluOpType.mult, op1=mybir.AluOpType.add)
                rz0 = rz_new
                rzrec = wp.tile([K, 1], F32, tag="rzrec", name="rzrec")
                nc.vector.reciprocal(rzrec, rz0)

            # ------- output: C = Y Mx^T = Yneg (-Mx^T) ----------------------
            yt_p = ps_tile([K, K])
            nc.tensor.transpose(yt_p, yn_s, id64)
            yt_s = wp.tile([K, K], F32, tag="yt_s", name="yt_s")
            nc.vector.tensor_copy(yt_s, yt_p)
            c_p = ps_tile([K, K])
            nc.tensor.matmul(c_p, yt_s, mxTn_s)     # Yneg (-Mx^T)
            c_s = wp.tile([K, K], F32, tag="c_s", name="c_s")
            nc.vector.tensor_copy(c_s, c_p)
            nc.sync.dma_start(out_d[:, :], c_s)

    nc.finalize()
    return nc


def get_program(shard: bool = False, debug: bool = False):
    key = ("prog", debug)
    if key not in _PROGRAM_CACHE:
        _PROGRAM_CACHE[key] = build_program(shard, debug=debug)
    return _PROGRAM_CACHE[key]


def make_in_maps(inputs, shard: bool = False):
    bf = ml_dtypes.bfloat16
    fx = np.ascontiguousarray(np.asarray(inputs["feat_x"], np.float32)[0]).astype(
        bf).reshape(CHUNK, NCH * C)
    fy = np.ascontiguousarray(np.asarray(inputs["feat_y"], np.float32)[0]).astype(
        bf).reshape(CHUNK, NCH * C)
    pxT = np.ascontiguousarray(
        np.asarray(inputs["evecs_trans_x"], np.float32)[0].T).astype(
        bf).reshape(CHUNK, NCH * K)
    pyT = np.ascontiguousarray(
        np.asarray(inputs["evecs_trans_y"], np.float32)[0].T).astype(
        bf).reshape(CHUNK, NCH * K)
    mx = np.ascontiguousarray(np.asarray(inputs["sqrtMk_x"], np.float32)[0])
    my = np.ascontiguousarray(np.asarray(inputs["sqrtMk_y"], np.float32)[0])
    ev = np.ascontiguousarray(np.concatenate([
        np.asarray(inputs["evals_x"], np.float32)[0],
        np.asarray(inputs["evals_y"], np.float32)[0],
    ])[None, :])
    # packed smalls: my | mx | I | -Mx^T   (negated: y is accumulated as -y)
    small = np.ascontiguousarray(np.concatenate(
        [my, mx, np.eye(64, dtype=np.float32), -mx.T], axis=1))
    m = {
        "fx": fx, "fy": fy, "pxT": pxT, "pyT": pyT,
        "small": small, "ev": ev,
    }
    return [dict(m) for _ in range(N_CORES)]


def kernel(**inputs) -> np.ndarray:
    nc = get_program(SHARD)
    in_maps = make_in_maps(inputs, SHARD)
    res = run_bass_kernel_spmd(nc, in_maps, core_ids=list(range(N_CORES)))
    out = np.asarray(res.results[0]["out"], dtype=np.float32)
    return out[None]
